# revision 1
# baseline (speedup 1.0000x reference)
"""MoE BaseLayer kernel for 8 Trainium2 NeuronCores.

Strategy (expert-parallel, per the sharding hint):
  * Host computes the top-1 routing (argmax of x @ centroids.T) and the sigmoid
    gate for the assigned expert -- this IS the sharding decision: tokens are
    gathered per-expert ("all-to-all dispatch" done host-side since kernel()
    receives full inputs) and each of the 8 cores gets one expert's tokens,
    padded to the max per-expert count C.
  * Each core runs, for its tokens: LayerNorm -> per-expert affine ->
    relu(xn @ w1.T + b1) -> @ w2.T + b2 -> out = x + a * ffn   (bf16 matmuls,
    fp32 accumulation; everything in a D-major [D, tokens] layout so no
    on-device transposes are needed).
  * Host scatters per-expert outputs back to token order.

Device layout notes:
  * x is kept D-major: [128 partitions, D/128 chunks, C tokens].
  * LN statistics (sum, sum of squares) via ones-vector matmuls (contraction
    over partitions); per-token mu/rstd broadcast back across partitions with
    rank-1 (K=1) matmuls against a ones row.
  * mm1: psum[f_chunk(128), N] += w1T[d_chunk, f_cols].T @ xn[d_chunk, N]
  * mm2: psum[d_chunk(128), N] += w2T[f_chunk, d_cols].T @ h[f_chunk, N]
  * w1 (bf16, 8MB) stays resident in SBUF; w2 is streamed per d-chunk slab.
"""

import sys

if "/opt/trn_rl_repo" not in sys.path:
    sys.path.insert(0, "/opt/trn_rl_repo")

import math

import ml_dtypes
import numpy as np

P = 128
D = 1024
F = 4096
E = 8
DC = D // P
FC = F // P
NCORES = 8
LN_EPS = 1e-5
BF16 = ml_dtypes.bfloat16

_compiled = {}


def _ensure_ntff_hook():
    """run_bass_kernel_spmd(trace=True) imports antenv.axon_hooks, which this
    container's antenv package lacks -- register the profiling hook via the
    libaxon_pjrt.so C ABI (mirrors trn_agent_boot.trn_boot) so tracing works
    instead of raising. No-op when the real module exists."""
    try:
        import antenv.axon_hooks  # noqa: F401

        return
    except ImportError:
        pass
    import contextlib
    import ctypes
    import types

    try:
        lib = ctypes.CDLL("/opt/axon/libaxon_pjrt.so")
        if not hasattr(lib, "axon_start_nrt_profile"):
            raise OSError("no profile ABI")
        lib.axon_start_nrt_profile.argtypes = [
            ctypes.POINTER(ctypes.c_int64),
            ctypes.c_size_t,
        ]
        lib.axon_start_nrt_profile.restype = ctypes.c_int64
        lib.axon_stop_nrt_profile.argtypes = [ctypes.c_char_p]
        lib.axon_stop_nrt_profile.restype = ctypes.c_int64

        @contextlib.contextmanager
        def _hook(output_dir, device_ids):
            import jax

            jax.devices()
            if device_ids:
                ids = (ctypes.c_int64 * len(device_ids))(*device_ids)
                rc = lib.axon_start_nrt_profile(ids, len(device_ids))
            else:
                rc = lib.axon_start_nrt_profile(None, 0)
            if rc != 0:
                raise RuntimeError(f"axon_start_nrt_profile rc={rc}")
            try:
                yield
            finally:
                lib.axon_stop_nrt_profile(str(output_dir).encode())

        get_hook = lambda: _hook  # noqa: E731
    except OSError:
        get_hook = lambda: None  # noqa: E731

    mod = types.ModuleType("antenv.axon_hooks")
    mod.get_axon_ntff_profile_hook = get_hook
    mod.set_axon_ntff_profile_hook = lambda h: None
    sys.modules["antenv.axon_hooks"] = mod
    try:
        import antenv

        antenv.axon_hooks = mod
    except ImportError:
        pass


def _token_tiles(C):
    """Token tile sizes <= 512: a small first tile to prime the DMA/LN
    pipeline, the rest balanced."""
    if C <= 512:
        return [(0, C)]
    first = 256
    rest = C - first
    nt = max(1, math.ceil(rest / 512))
    base = rest // nt
    rem = rest % nt
    sizes = [first] + [base + (1 if i < rem else 0) for i in range(nt)]
    tiles = []
    s = 0
    for n in sizes:
        tiles.append((s, n))
        s += n
    return tiles


def _build(C):
    import concourse.tile as tile
    from concourse import bacc, mybir

    f32 = mybir.dt.float32
    bf16 = mybir.dt.bfloat16
    AO = mybir.AluOpType
    AF = mybir.ActivationFunctionType

    tiles = _token_tiles(C)
    NMAX = max(n for _, n in tiles)

    nc = bacc.Bacc("TRN2", target_bir_lowering=False, debug=False)

    FG = F // 8  # w1 f-column group size
    xT = nc.dram_tensor("xT", (D, C), f32, kind="ExternalInput").ap()
    gate = nc.dram_tensor("gate", (1, C), f32, kind="ExternalInput").ap()
    w1r = nc.dram_tensor("w1r", (8, P, DC * FG), bf16, kind="ExternalInput").ap()
    w2s = nc.dram_tensor("w2s", (DC, P, FC * P), bf16, kind="ExternalInput").ap()
    b1r = nc.dram_tensor("b1r", (P, FC), f32, kind="ExternalInput").ap()
    b2r = nc.dram_tensor("b2r", (P, DC), f32, kind="ExternalInput").ap()
    outT = nc.dram_tensor("outT", (D, C), f32, kind="ExternalOutput").ap()

    xv = xT.rearrange("(c p) n -> p c n", p=P)
    ov = outT.rearrange("(c p) n -> p c n", p=P)

    with tile.TileContext(nc) as tc:
        with (
            tc.tile_pool(name="wres", bufs=1) as wres,
            tc.tile_pool(name="w2p", bufs=4) as w2p,
            tc.tile_pool(name="cst", bufs=1) as cst,
            tc.tile_pool(name="big", bufs=1) as big,
            tc.tile_pool(name="xsp", bufs=2) as xsp,
            tc.tile_pool(name="sqp", bufs=2) as sqp,
            tc.tile_pool(name="xbp", bufs=2) as xbp,
            tc.tile_pool(name="ltp", bufs=2) as ltp,
            tc.tile_pool(name="ctp", bufs=2) as ctp,
            tc.tile_pool(name="otp", bufs=2) as otp,
            tc.tile_pool(name="pstat", bufs=1, space="PSUM") as pstat,
            tc.tile_pool(name="prep", bufs=2, space="PSUM") as prep,
            tc.tile_pool(name="php", bufs=2, space="PSUM") as php,
            tc.tile_pool(name="pyp", bufs=2, space="PSUM") as pyp,
        ):
            # ---- DMA ordering on the SP ring: first x slice, then the small
            # constants, then the w1 groups (128-row 8KB-contiguous slabs so
            # descriptor generation is cheap), then the remaining x slices.
            # Nothing bulk goes on the ACT ring -- its queue carries the
            # latency-critical squares/copies/relu.
            x_tiles = [
                xsp.tile([P, DC, NMAX], f32, tag="xs", name=f"xs{i}")[:, :, :N]
                for i, (_, N) in enumerate(tiles)
            ]
            # first slice in two halves: stats on chunks 0-3 start ~3us sooner,
            # bridging the warmup dummies to dense matmul work (keeps HAM warm)
            nc.sync.dma_start(x_tiles[0][:, 0 : DC // 2, :], xv[:, 0 : DC // 2, 0 : tiles[0][1]])
            nc.sync.dma_start(x_tiles[0][:, DC // 2 :, :], xv[:, DC // 2 :, 0 : tiles[0][1]])

            b1_sb = cst.tile([P, FC], f32)
            nc.sync.dma_start(b1_sb[:], b1r)
            b2_sb = cst.tile([P, DC], f32)
            nc.sync.dma_start(b2_sb[:], b2r)
            a_sb = cst.tile([1, C], f32)
            nc.sync.dma_start(a_sb[:], gate)
            ones_col = cst.tile([P, 1], bf16)
            nc.vector.memset(ones_col[:], 1.0 / D)  # 2^-10, exact in bf16
            ones_row_bf = cst.tile([1, P], bf16)
            nc.vector.memset(ones_row_bf[:], 1.0)
            eps_sb = cst.tile([1, 1], f32)
            nc.vector.memset(eps_sb[:], LN_EPS)

            # HAM warmup: cheap bf16 dummy matmuls during the initial x DMA
            # wait, so the PE clock gate is open when the real work starts.
            WN = min(256, NMAX)
            scr_bf = cst.tile([P, WN], bf16)
            nc.vector.memset(scr_bf[:], 0.0)
            psw = pstat.tile([1, NMAX], f32, tag="ps0", name="psw")[:, :WN]
            for _ in range(16):
                nc.tensor.matmul(psw, ones_col[:], scr_bf[:], start=True, stop=True)

            # w1 as 8 separate group tiles so mm1 only waits on the group it reads
            w1v = w1r.rearrange("g p (c j) -> g p c j", c=DC)
            w1g = []
            for fg in range(8):
                wt = wres.tile([P, DC, FG], bf16, name=f"w1g{fg}")
                nc.sync.dma_start(wt[:], w1v[fg])
                w1g.append(wt)

            ah_sb = big.tile([1, C], bf16)
            al_sb = big.tile([1, C], bf16)
            mub_sb = big.tile([1, C], bf16)
            rsb_sb = big.tile([1, C], bf16)
            s0_sb = big.tile([1, C], f32)  # sum -> mu
            s1_sb = big.tile([1, C], f32)  # sumsq -> var -> rstd
            nc.vector.tensor_copy(ah_sb[:], a_sb[:])
            for s0_ in range(0, C, NMAX):
                w_ = min(NMAX, C - s0_)
                atmp = ltp.tile([P, NMAX], f32, tag="lt", name="atmp")[0:1, :w_]
                nc.vector.tensor_sub(atmp, a_sb[:, s0_ : s0_ + w_], ah_sb[:, s0_ : s0_ + w_])
                nc.vector.tensor_copy(al_sb[:, s0_ : s0_ + w_], atmp)

            repa_sb = big.tile([P, C], f32)
            xn_sb = big.tile([P, DC, C], bf16)
            h_sb = big.tile([P, FC, NMAX], bf16)

            # ---- prologue (per token slice), in two parts: A = stats matmuls
            # + mu/rstd chain; B = partition-broadcasts + normalize. B is
            # emitted well after A so the PE never waits on the DVE chain. ----
            def prologue_stats(ti):
                S, N = tiles[ti]
                sl = slice(S, S + N)
                x_s = x_tiles[ti]

                ps0 = pstat.tile([1, NMAX], f32, tag="ps0", name="ps0")[:, :N]
                ps1 = pstat.tile([1, NMAX], f32, tag="ps1", name="ps1")[:, :N]
                for c in range(DC):
                    xbt = xbp.tile([P, NMAX], bf16, tag="xb", name="xb")[:, :N]
                    nc.vector.tensor_copy(xbt, x_s[:, c, :])
                    nc.tensor.matmul(
                        ps0, ones_col[:], xbt, start=(c == 0), stop=(c == DC - 1)
                    )
                    sqt = sqp.tile([P, NMAX], bf16, tag="sq", name="sq")[:, :N]
                    nc.vector.tensor_mul(sqt, xbt, xbt)
                    nc.tensor.matmul(
                        ps1, ones_col[:], sqt, start=(c == 0), stop=(c == DC - 1)
                    )

                nc.vector.tensor_copy(s0_sb[:, sl], ps0)
                nc.vector.tensor_copy(s1_sb[:, sl], ps1)
                m2t = ltp.tile([P, NMAX], f32, tag="lt", name="m2t")[0:1, :N]
                nc.vector.tensor_mul(m2t, s0_sb[:, sl], s0_sb[:, sl])
                nc.vector.tensor_sub(s1_sb[:, sl], s1_sb[:, sl], m2t)
                nc.scalar.activation(
                    s1_sb[:, sl], s1_sb[:, sl], AF.Sqrt, bias=eps_sb[:, 0:1]
                )
                nc.vector.reciprocal_approx_fast(s1_sb[:, sl], s1_sb[:, sl])
                nc.vector.tensor_copy(mub_sb[:, sl], s0_sb[:, sl])
                nc.vector.tensor_copy(rsb_sb[:, sl], s1_sb[:, sl])

            def prologue_ln(ti):
                S, N = tiles[ti]
                sl = slice(S, S + N)
                x_s = x_tiles[ti]

                rmu = prep.tile([P, NMAX], f32, tag="rep", name="rep")[:, :N]
                nc.tensor.matmul(rmu, ones_row_bf[:], mub_sb[:, sl], start=True, stop=True)
                rrs = prep.tile([P, NMAX], f32, tag="rep", name="rep")[:, :N]
                nc.tensor.matmul(rrs, ones_row_bf[:], rsb_sb[:, sl], start=True, stop=True)

                # ln_g/ln_b are folded into w1/b1 on the host, so xn is just
                # the bf16 cast of xhat = (x - mu) * rstd.
                for c in range(DC):
                    t1 = ltp.tile([P, NMAX], f32, tag="lt", name="lt")[:, :N]
                    nc.vector.tensor_sub(t1, x_s[:, c, :], rmu)
                    nc.vector.tensor_mul(xn_sb[:, c, sl], t1, rrs)

                ra = prep.tile([P, NMAX], f32, tag="rep", name="rep")[:, :N]
                nc.tensor.matmul(ra, ones_row_bf[:], ah_sb[:, sl], start=True, stop=False)
                nc.tensor.matmul(ra, ones_row_bf[:], al_sb[:, sl], start=False, stop=True)
                nc.scalar.copy(repa_sb[:, sl], ra)

            prologue_stats(0)
            # fill the PE wait for the slice-0 LN chain with dummies so the
            # clock gate stays open into the first matmul tile
            psw2 = pstat.tile([1, NMAX], f32, tag="ps0", name="psw2")[:, :WN]
            for _ in range(24):
                nc.tensor.matmul(psw2, ones_col[:], scr_bf[:], start=True, stop=True)
            prologue_ln(0)

            # ---- main loop: FFN per token tile; next slice's prologue is
            # emitted after this tile's mm1 so its DVE work hides under the
            # matmuls instead of gating them ----
            for ti, (S, N) in enumerate(tiles):
                sl = slice(S, S + N)
                if ti + 1 < len(tiles):
                    S2, N2 = tiles[ti + 1]
                    nc.sync.dma_start(x_tiles[ti + 1], xv[:, :, S2 : S2 + N2])
                for f in range(FC):
                    ph = php.tile([P, NMAX], f32, tag="ph", name="ph")[:, :N]
                    wg = w1g[f // 4]
                    fo = f % 4
                    for c in range(DC):
                        nc.tensor.matmul(
                            ph,
                            wg[:, c, fo * P : (fo + 1) * P],
                            xn_sb[:, c, sl],
                            start=(c == 0),
                            stop=(c == DC - 1),
                        )
                    nc.scalar.activation(
                        h_sb[:, f, :N], ph, AF.Relu, bias=b1_sb[:, f : f + 1]
                    )

                if ti + 1 < len(tiles):
                    prologue_stats(ti + 1)

                last_tile = ti == len(tiles) - 1
                for d in range(DC):
                    if d == 4 and not last_tile:
                        prologue_ln(ti + 1)
                    w2t = w2p.tile([P, FC * P], bf16, tag="w2")
                    nc.sync.dma_start(w2t[:], w2s[d])

                    # split the very last d-group so its combine/store overlaps
                    # the final matmuls instead of trailing the kernel
                    if last_tile and d == DC - 1:
                        halves = [(0, N // 2), (N // 2, N - N // 2)]
                    else:
                        halves = [(0, N)]
                    for h0, hn in halves:
                        hsl = slice(S + h0, S + h0 + hn)
                        py = pyp.tile([P, NMAX], f32, tag="py", name="py")[:, :hn]
                        for fi in range(FC):
                            nc.tensor.matmul(
                                py,
                                w2t[:, fi * P : (fi + 1) * P],
                                h_sb[:, fi, h0 : h0 + hn],
                                start=(fi == 0),
                                stop=(fi == FC - 1),
                            )
                        tcm = ctp.tile([P, NMAX], f32, tag="ct", name="ct")[:, :hn]
                        nc.scalar.add(tcm, py, b2_sb[:, d : d + 1])
                        nc.vector.tensor_mul(tcm, tcm, repa_sb[:, hsl])
                        ot = otp.tile([P, NMAX], f32, tag="ot", name="ot")[:, :hn]
                        nc.vector.tensor_add(
                            ot, tcm, x_tiles[ti][:, d, h0 : h0 + hn]
                        )
                        if last_tile:
                            nc.sync.dma_start(ov[:, d, hsl], ot)
                        else:
                            nc.gpsimd.dma_start(ov[:, d, hsl], ot)

    nc.compile()
    return nc


def _get_compiled(C):
    if C not in _compiled:
        _compiled[C] = _build(C)
    return _compiled[C]


def _prep(inputs):
    x = np.ascontiguousarray(
        np.asarray(inputs["input_features"], np.float32).reshape(-1, D)
    )
    T = x.shape[0]
    cent = np.asarray(inputs["centroids"], np.float64)
    w1 = np.asarray(inputs["w1"], np.float32)
    b1 = np.asarray(inputs["b1"], np.float32)
    w2 = np.asarray(inputs["w2"], np.float32)
    b2 = np.asarray(inputs["b2"], np.float32)
    ln_g = np.asarray(inputs["ln_g"], np.float32)
    ln_b = np.asarray(inputs["ln_b"], np.float32)

    aff = x.astype(np.float64) @ cent.T
    assign = aff.argmax(1)
    alpha = 1.0 / (1.0 + np.exp(-aff[np.arange(T), assign]))

    counts = np.bincount(assign, minlength=E)
    C = max(int(counts.max()), P)

    idx_list = []
    in_maps = []
    for e in range(NCORES):
        idx = np.nonzero(assign == e)[0]
        cnt = len(idx)
        idx_list.append(idx)

        xTb = np.zeros((D, C), np.float32)
        xTb[:, :cnt] = x[idx].T
        gate_e = np.zeros((1, C), np.float32)
        gate_e[0, :cnt] = alpha[idx]

        # fold the per-expert LN affine into w1/b1:
        #   relu((xh*g + b) @ w1.T + b1) = relu(xh @ (w1*g).T + (b1 + w1 @ b))
        w1f = w1[e] * ln_g[e][None, :]
        b1f = b1[e] + w1[e] @ ln_b[e]
        w1T = w1f.T  # [D, F]
        FG = F // 8
        w1rb = np.ascontiguousarray(
            w1T.reshape(DC, P, 8, FG).transpose(2, 1, 0, 3).reshape(8, P, DC * FG)
        ).astype(BF16)
        w2T = w2[e].T  # [F, D]
        w2sb = np.ascontiguousarray(
            w2T.reshape(FC, P, DC, P).transpose(2, 1, 0, 3).reshape(DC, P, FC * P)
        ).astype(BF16)

        in_maps.append(
            {
                "xT": xTb,
                "gate": gate_e,
                "w1r": w1rb,
                "w2s": w2sb,
                "b1r": np.ascontiguousarray(b1f.reshape(FC, P).T),
                "b2r": np.ascontiguousarray(b2[e].reshape(DC, P).T),
            }
        )
    return C, idx_list, in_maps, T


def _run(inputs, trace=False, trace_cores=None, stitch_traces=False):
    _ensure_ntff_hook()
    from concourse.bass_utils import run_bass_kernel_spmd

    C, idx_list, in_maps, T = _prep(inputs)
    nc = _get_compiled(C)
    res = run_bass_kernel_spmd(
        nc,
        in_maps,
        core_ids=list(range(NCORES)),
        trace=trace,
        trace_cores=trace_cores,
        stitch_traces=stitch_traces,
    )
    out = np.zeros((T, D), np.float32)
    for e in range(NCORES):
        idx = idx_list[e]
        out[idx] = res.results[e]["outT"][:, : len(idx)].T
    out = out.reshape(np.asarray(inputs["input_features"]).shape)
    return out, res


def kernel(**inputs):
    out, _ = _run(inputs)
    return out

